# revision 15
# baseline (speedup 1.0000x reference)
"""Trainium2 Bass kernel for nn_DotAttention (B=4, NH=16, L=S=2048, H=64).

Architecture (per core; 64 (b,nh) pairs sharded 8-per-core over 8 cores):
  - Host: cast Q,K,V to bf16; build augmented transposed operands
      QT[128, L]: rows 0..63 = Q^T, row 64 = sqrt(H), rows 65.. = 0
      KT[128, S]: rows 0..63 = K^T, row 64 = -1e8 * mask[b,s], rest 0
    so the QK matmul yields  scores*sqrt(H) + (-1e8*sqrt(H))*mask  and the
    activation's scale=1/sqrt(H) turns that into  scores - 1e8*mask  — the
    masking costs zero extra device passes.
  - Device per head:
      QK:   psum[128l, S] = QT_lsub^T @ KT           (TensorE, bf16)
      exp:  E = exp(scale*psum)  (ScalarE, one pass) with accum_out giving
            the softmax denominators (row sums) for free.
      W    = E * (head_mask / sum)   (VectorE tensor_scalar, per-partition)
      DMA W (bf16) -> HBM  == the attention_weight output (upcast on host)
      readback: one 4MB xbar transpose-DMA per half-head brings W back as
            W^T tiles [128s, NS, l] (source rows are full contiguous W rows)
      PV:   attn_out[128l, H] += W^T_tile^T @ V_tile  (TensorE) — comes out
            already normalized since W is normalized.
"""

import sys

sys.path.insert(0, "/opt/trn_rl_repo")

import numpy as np
import ml_dtypes

import concourse.bass as bass
import concourse.bacc as bacc
import concourse.mybir as mybir
import concourse.tile as tile
from concourse.tile_rust import add_dep_helper
from concourse.bass_utils import run_bass_kernel_spmd

BF16 = mybir.dt.bfloat16
F32 = mybir.dt.float32
N_CORES = 8
NEG = -100000000.0  # mask fill value (matches reference)
P = 128


def emit_attention(tc, qt, kt, v, hm, w_out, attn_out, HPC, L, S, H):
    nc = tc.nc
    NL = L // P  # l subtiles per head
    NS = S // P  # s subtiles per head
    NPC = 2 if S >= 1024 else 1  # qk psum tiles per l-sub (double buffering)
    SPC = S // NPC  # s-range covered by one qk psum tile
    QCH = min(512, SPC)  # qk matmul free-dim chunk (one psum bank)
    scale = float(1.0 / np.sqrt(H))
    NLH = NL // 2  # l subtiles per half (readback granularity)
    L2 = L // 2
    NRB = 4 if NL >= 8 else 2  # readback slabs per head
    LRB = L // NRB  # l-range per readback slab
    NLR = NL // NRB  # l subtiles per readback slab
    WB = min(2, NLR)  # l-subs per W write batch
    OB = min(4, NLR)  # l-subs per attn_out write batch
    assert NLR % WB == 0 and NLR % OB == 0

    import contextlib

    with contextlib.ExitStack() as ctx:
        io_pool = ctx.enter_context(tc.tile_pool(name="io", bufs=3))
        e_pool = ctx.enter_context(tc.tile_pool(name="e", bufs=3))
        w_pool = ctx.enter_context(tc.tile_pool(name="w", bufs=2))
        wt_pool = ctx.enter_context(tc.tile_pool(name="wt", bufs=2))
        st_pool = ctx.enter_context(tc.tile_pool(name="st", bufs=8))
        o_pool = ctx.enter_context(tc.tile_pool(name="o", bufs=2))
        cst_pool = ctx.enter_context(tc.tile_pool(name="cst", bufs=1))
        qk_psum = ctx.enter_context(tc.tile_pool(name="qkp", bufs=2, space="PSUM"))
        pv_psum = ctx.enter_context(tc.tile_pool(name="pvp", bufs=3, space="PSUM"))

        # head_mask for all heads, partition-major: hm[p, h]
        hm_sb = cst_pool.tile([P, HPC], F32, tag="hm")
        nc.sync.dma_start(hm_sb[:], hm[:])

        for h in range(HPC):
            qt_sb = io_pool.tile([P, L], BF16, tag="qt")
            nc.sync.dma_start(qt_sb[:], qt[h])
            kt_sb = io_pool.tile([P, S], BF16, tag="kt")
            nc.sync.dma_start(kt_sb[:], kt[h])
            # v param is pre-shuffled on host so this load is contiguous:
            # v[h] is [128, NS*H] with v[h][p, si*H:] = V[si*128+p, :]
            v_sb = io_pool.tile([P, NS, H], BF16, tag="v")
            nc.sync.dma_start(v_sb[:], v[h].rearrange("p (si x) -> p si x", si=NS))

            w_write_insts = []
            w4 = None
            for i in range(NL):
                e_i = e_pool.tile([P, S], BF16, tag="e")
                sum_parts = []
                for sc in range(NPC):
                    psq = qk_psum.tile([P, SPC], F32, tag="qk")
                    for j in range(SPC // QCH):
                        nc.tensor.matmul(
                            psq[:, j * QCH : (j + 1) * QCH],
                            lhsT=qt_sb[:, i * P : (i + 1) * P],
                            rhs=kt_sb[
                                :, sc * SPC + j * QCH : sc * SPC + (j + 1) * QCH
                            ],
                            start=True,
                            stop=True,
                        )
                    spart = st_pool.tile([P, 1], F32, tag=f"sumpart{sc}")
                    nc.scalar.activation(
                        e_i[:, sc * SPC : (sc + 1) * SPC],
                        psq[:],
                        mybir.ActivationFunctionType.Exp,
                        scale=scale,
                        accum_out=spart[:],
                    )
                    sum_parts.append(spart)
                if NPC == 2:
                    sums = st_pool.tile([P, 1], F32, tag="sums")
                    nc.vector.tensor_add(sums[:], sum_parts[0][:], sum_parts[1][:])
                else:
                    sums = sum_parts[0]
                recip = st_pool.tile([P, 1], F32, tag="recip")
                nc.vector.reciprocal(recip[:], sums[:])
                receff = st_pool.tile([P, 1], F32, tag="receff")
                nc.vector.tensor_mul(receff[:], recip[:], hm_sb[:, h : h + 1])
                if i % WB == 0:
                    w4 = w_pool.tile([P, WB, S], BF16, tag="w")
                nc.vector.tensor_scalar_mul(w4[:, i % WB, :], e_i[:], receff[:])
                if i % WB == WB - 1:
                    blk = i // WB
                    wi = nc.sync.dma_start(
                        w_out[h, blk * WB * P : (blk + 1) * WB * P, :].rearrange(
                            "(a p) s -> p a s", p=P
                        ),
                        w4[:],
                    )
                    w_write_insts.append(wi)

            # Read W back transposed: one xbar transpose-DMA per L-slab.
            # Source rows are full W rows (4KB contiguous).
            for rb in range(NRB):
                wt_h = wt_pool.tile([P, NS, LRB], BF16, tag="wt")
                rd = nc.sync.dma_start_transpose(
                    wt_h[:],
                    w_out[h, rb * LRB : (rb + 1) * LRB, :],
                )
                for wi in w_write_insts[
                    rb * (NLR // WB) : (rb + 1) * (NLR // WB)
                ]:
                    add_dep_helper(rd.ins, wi.ins, reason="W readback after write")
                o4 = None
                for ii in range(NLR):
                    i = rb * NLR + ii
                    psv = pv_psum.tile([P, H], F32, tag="pv")
                    for si in range(NS):
                        nc.tensor.matmul(
                            psv[:],
                            lhsT=wt_h[:, si, ii * P : (ii + 1) * P],
                            rhs=v_sb[:, si, :],
                            start=(si == 0),
                            stop=(si == NS - 1),
                        )
                    if ii % OB == 0:
                        o4 = o_pool.tile([P, OB, H], F32, tag="osb")
                    nc.vector.tensor_copy(o4[:, ii % OB, :], psv[:])
                    if ii % OB == OB - 1:
                        blk = i // OB
                        nc.sync.dma_start(
                            attn_out[
                                h, blk * OB * P : (blk + 1) * OB * P, :
                            ].rearrange("(a p) x -> p a x", p=P),
                            o4[:],
                        )


def build_nc(HPC, L, S, H, n_cores=N_CORES):
    nc = bacc.Bacc(
        "TRN2", target_bir_lowering=False, debug=False, num_devices=n_cores
    )
    qt = nc.declare_dram_parameter("qt", [HPC, P, L], BF16, isOutput=False)
    kt = nc.declare_dram_parameter("kt", [HPC, P, S], BF16, isOutput=False)
    v = nc.declare_dram_parameter("v", [HPC, P, (S // P) * H], BF16, isOutput=False)
    hm = nc.declare_dram_parameter("hm", [P, HPC], F32, isOutput=False)
    w_out = nc.declare_dram_parameter("w_out", [HPC, L, S], BF16, isOutput=True)
    attn_out = nc.declare_dram_parameter("attn_out", [HPC, L, H], F32, isOutput=True)
    with tile.TileContext(nc) as tc:
        emit_attention(tc, qt, kt, v, hm, w_out, attn_out, HPC, L, S, H)
    nc.compile()
    return nc


def make_in_maps(Q, K, V, mask_out, head_mask, n_cores=N_CORES):
    bf16 = ml_dtypes.bfloat16
    B, NH, L, H = Q.shape
    S = K.shape[2]
    NS = S // P
    NHtot = B * NH
    HPC = NHtot // n_cores
    sH = np.float32(np.sqrt(H))

    QT = np.zeros((B, NH, P, L), dtype=bf16)
    QT[:, :, :H, :] = Q.transpose(0, 1, 3, 2)
    QT[:, :, H, :] = sH
    KT = np.zeros((B, NH, P, S), dtype=bf16)
    KT[:, :, :H, :] = K.transpose(0, 1, 3, 2)
    maskrow = np.where(
        np.asarray(mask_out).reshape(B, 1, S), np.float32(NEG), np.float32(0.0)
    )
    KT[:, :, H, :] = maskrow  # broadcast over NH
    # V shuffled so the device tile [128, NS, H] loads contiguously:
    # vdev[h, p, si*H + x] = V[h, si*128 + p, x]
    Vb = np.asarray(V).astype(bf16).reshape(NHtot, NS, P, H)
    Vdev = np.ascontiguousarray(Vb.transpose(0, 2, 1, 3)).reshape(
        NHtot, P, NS * H
    )
    hm_full = np.ascontiguousarray(
        np.broadcast_to(
            np.asarray(head_mask, dtype=np.float32).reshape(1, NHtot), (P, NHtot)
        )
    )

    QTf = QT.reshape(NHtot, P, L)
    KTf = KT.reshape(NHtot, P, S)
    in_maps = []
    for c in range(n_cores):
        sl = slice(c * HPC, (c + 1) * HPC)
        in_maps.append(
            {
                "qt": np.ascontiguousarray(QTf[sl]),
                "kt": np.ascontiguousarray(KTf[sl]),
                "v": np.ascontiguousarray(Vdev[sl]),
                "hm": np.ascontiguousarray(hm_full[:, sl]),
            }
        )
    return in_maps, HPC


def assemble_outputs(results, B, NH, L, S, H):
    NHtot = B * NH
    n_cores = len(results)
    HPC = NHtot // n_cores
    attention_weight = np.empty((NHtot, L, S), dtype=np.float32)
    attention_out = np.empty((NHtot, L, H), dtype=np.float32)
    for c, out in enumerate(results):
        sl = slice(c * HPC, (c + 1) * HPC)
        attention_weight[sl] = np.asarray(out["w_out"]).astype(np.float32)
        attention_out[sl] = np.asarray(out["attn_out"]).astype(np.float32)
    return (
        attention_out.reshape(B, NH, L, H),
        attention_weight.reshape(B, NH, L, S),
    )


def run(Q, K, V, mask_out, head_mask, trace=False):
    Q = np.asarray(Q, dtype=np.float32)
    K = np.asarray(K, dtype=np.float32)
    V = np.asarray(V, dtype=np.float32)
    B, NH, L, H = Q.shape
    S = K.shape[2]
    in_maps, HPC = make_in_maps(Q, K, V, mask_out, head_mask)
    nc = build_nc(HPC, L, S, H)
    res = run_bass_kernel_spmd(nc, in_maps, list(range(N_CORES)), trace=trace)
    out = assemble_outputs(res.results, B, NH, L, S, H)
    return out, res


def kernel(Q, K, V, mask_out, head_mask):
    out, _ = run(Q, K, V, mask_out, head_mask)
    return out


# revision 17
# speedup vs baseline: 1.1700x; 1.1700x over previous
"""Trainium2 Bass kernel for nn_DotAttention (B=4, NH=16, L=S=2048, H=64).

Architecture (per core; 64 (b,nh) pairs sharded 8-per-core over 8 cores):
  - Host: cast Q,K,V to bf16; build augmented transposed operands
      QT[128, L]: rows 0..63 = Q^T, row 64 = sqrt(H), rows 65.. = 0
      KT[128, S]: rows 0..63 = K^T, row 64 = -1e8 * mask[b,s], rest 0
    so the QK matmul yields  scores*sqrt(H) + (-1e8*sqrt(H))*mask  and the
    activation's scale=1/sqrt(H) turns that into  scores - 1e8*mask  — the
    masking costs zero extra device passes.
  - Device per head:
      QK:   psum[128l, S] = QT_lsub^T @ KT           (TensorE, bf16)
      exp:  E = exp(scale*psum)  (ScalarE, one pass) with accum_out giving
            the softmax denominators (row sums) for free.
      W    = E * (head_mask / sum)   (VectorE tensor_scalar, per-partition)
      DMA W (bf16) -> HBM  == the attention_weight output (upcast on host)
      readback: one 4MB xbar transpose-DMA per half-head brings W back as
            W^T tiles [128s, NS, l] (source rows are full contiguous W rows)
      PV:   attn_out[128l, H] += W^T_tile^T @ V_tile  (TensorE) — comes out
            already normalized since W is normalized.
"""

import sys

sys.path.insert(0, "/opt/trn_rl_repo")

import numpy as np
import ml_dtypes

import concourse.bass as bass
import concourse.bacc as bacc
import concourse.mybir as mybir
import concourse.tile as tile
from concourse.tile_rust import add_dep_helper
from concourse.bass_utils import run_bass_kernel_spmd

BF16 = mybir.dt.bfloat16
F32 = mybir.dt.float32
N_CORES = 8
NEG = -100000000.0  # mask fill value (matches reference)
P = 128
B_HEADS = 1  # trailing heads that compute PV via 2nd exp instead of readback


def emit_attention(tc, qt, kt, v, hm, w_out, attn_out, HPC, L, S, H):
    nc = tc.nc
    NL = L // P  # l subtiles per head
    NS = S // P  # s subtiles per head
    NPC = 2 if S >= 1024 else 1  # qk psum tiles per l-sub (double buffering)
    SPC = S // NPC  # s-range covered by one qk psum tile
    QCH = min(512, SPC)  # qk matmul free-dim chunk (one psum bank)
    scale = float(1.0 / np.sqrt(H))
    NLH = NL // 2  # l subtiles per half (readback granularity)
    L2 = L // 2
    NRB = 2  # readback slabs per head
    LRB = L // NRB  # l-range per readback slab
    NLR = NL // NRB  # l subtiles per readback slab
    WB = min(4, NLR)  # l-subs per W write batch
    OB = min(4, NLR)  # l-subs per attn_out write batch
    assert NLR % WB == 0 and NLR % OB == 0

    import contextlib

    with contextlib.ExitStack() as ctx:
        io_pool = ctx.enter_context(tc.tile_pool(name="io", bufs=3))
        e_pool = ctx.enter_context(tc.tile_pool(name="e", bufs=3))
        w_pool = ctx.enter_context(tc.tile_pool(name="w", bufs=2))
        wt_pool = ctx.enter_context(tc.tile_pool(name="wt", bufs=2))
        st_pool = ctx.enter_context(tc.tile_pool(name="st", bufs=8))
        o_pool = ctx.enter_context(tc.tile_pool(name="o", bufs=2))
        cst_pool = ctx.enter_context(tc.tile_pool(name="cst", bufs=1))
        eb_pool = ctx.enter_context(tc.tile_pool(name="eb", bufs=2))
        rc_pool = ctx.enter_context(tc.tile_pool(name="rc", bufs=2))
        qk_psum = ctx.enter_context(tc.tile_pool(name="qkp", bufs=2, space="PSUM"))
        pv_psum = ctx.enter_context(tc.tile_pool(name="pvp", bufs=2, space="PSUM"))
        qkb_psum = ctx.enter_context(tc.tile_pool(name="qkbp", bufs=2, space="PSUM"))

        # head_mask for all heads, partition-major: hm[p, h]
        hm_sb = cst_pool.tile([P, HPC], F32, tag="hm")
        nc.sync.dma_start(hm_sb[:], hm[:])

        for h in range(HPC):
            qt_sb = io_pool.tile([P, L], BF16, tag="qt")
            nc.sync.dma_start(qt_sb[:], qt[h])
            kt_sb = io_pool.tile([P, S], BF16, tag="kt")
            nc.sync.dma_start(kt_sb[:], kt[h])
            # v param is pre-shuffled on host so this load is contiguous:
            # v[h] is [128, NS*H] with v[h][p, si*H:] = V[si*128+p, :]
            v_sb = io_pool.tile([P, NS, H], BF16, tag="v")
            nc.sync.dma_start(v_sb[:], v[h].rearrange("p (si x) -> p si x", si=NS))

            receff_head = rc_pool.tile([P, NL], F32, tag="receff")
            w_write_insts = []
            w4 = None
            for i in range(NL):
                e_i = e_pool.tile([P, S], BF16, tag="e")
                sum_parts = []
                for sc in range(NPC):
                    psq = qk_psum.tile([P, SPC], F32, tag="qk")
                    for j in range(SPC // QCH):
                        nc.tensor.matmul(
                            psq[:, j * QCH : (j + 1) * QCH],
                            lhsT=qt_sb[:, i * P : (i + 1) * P],
                            rhs=kt_sb[
                                :, sc * SPC + j * QCH : sc * SPC + (j + 1) * QCH
                            ],
                            start=True,
                            stop=True,
                        )
                    spart = st_pool.tile([P, 1], F32, tag=f"sumpart{sc}")
                    nc.scalar.activation(
                        e_i[:, sc * SPC : (sc + 1) * SPC],
                        psq[:],
                        mybir.ActivationFunctionType.Exp,
                        scale=scale,
                        accum_out=spart[:],
                    )
                    sum_parts.append(spart)
                if NPC == 2:
                    sums = st_pool.tile([P, 1], F32, tag="sums")
                    nc.vector.tensor_add(sums[:], sum_parts[0][:], sum_parts[1][:])
                else:
                    sums = sum_parts[0]
                recip = st_pool.tile([P, 1], F32, tag="recip")
                nc.vector.reciprocal(recip[:], sums[:])
                nc.vector.tensor_mul(
                    receff_head[:, i : i + 1], recip[:], hm_sb[:, h : h + 1]
                )
                if i % WB == 0:
                    w4 = w_pool.tile([P, WB, S], BF16, tag="w")
                nc.vector.tensor_scalar_mul(
                    w4[:, i % WB, :], e_i[:], receff_head[:, i : i + 1]
                )
                if i % WB == WB - 1:
                    blk = i // WB
                    wi = nc.sync.dma_start(
                        w_out[h, blk * WB * P : (blk + 1) * WB * P, :].rearrange(
                            "(a p) s -> p a s", p=P
                        ),
                        w4[:],
                    )
                    w_write_insts.append(wi)

            if h >= HPC - B_HEADS:
                # Second-exp path: recompute scores transposed (swap lhsT/rhs
                # -- zero new operands), exp on ScalarE, PV from SBUF. No
                # W readback DMA for this head.
                LCW = min(512, L)
                for lc in range(L // LCW):
                    eb = eb_pool.tile([P, NS, LCW], BF16, tag="eb")
                    for si in range(NS):
                        psb = qkb_psum.tile([P, LCW], F32, tag="qkb")
                        nc.tensor.matmul(
                            psb[:],
                            lhsT=kt_sb[:, si * P : (si + 1) * P],
                            rhs=qt_sb[:, lc * LCW : (lc + 1) * LCW],
                            start=True,
                            stop=True,
                        )
                        nc.scalar.activation(
                            eb[:, si, :],
                            psb[:],
                            mybir.ActivationFunctionType.Exp,
                            scale=scale,
                        )
                    o4 = None
                    NQL = LCW // P
                    for q in range(NQL):
                        i = lc * NQL + q
                        psv = pv_psum.tile([P, H], F32, tag="pv")
                        for si in range(NS):
                            nc.tensor.matmul(
                                psv[:],
                                lhsT=eb[:, si, q * P : (q + 1) * P],
                                rhs=v_sb[:, si, :],
                                start=(si == 0),
                                stop=(si == NS - 1),
                            )
                        if i % OB == 0:
                            o4 = o_pool.tile([P, OB, H], F32, tag="osb")
                        nc.vector.tensor_scalar_mul(
                            o4[:, i % OB, :], psv[:], receff_head[:, i : i + 1]
                        )
                        if i % OB == OB - 1:
                            blk = i // OB
                            nc.sync.dma_start(
                                attn_out[
                                    h, blk * OB * P : (blk + 1) * OB * P, :
                                ].rearrange("(a p) x -> p a x", p=P),
                                o4[:],
                            )
                continue

            # Read W back transposed: one xbar transpose-DMA per L-slab.
            # Source rows are full W rows (4KB contiguous).
            for rb in range(NRB):
                wt_h = wt_pool.tile([P, NS, LRB], BF16, tag="wt")
                rd = nc.sync.dma_start_transpose(
                    wt_h[:],
                    w_out[h, rb * LRB : (rb + 1) * LRB, :],
                )
                for wi in w_write_insts[
                    rb * (NLR // WB) : (rb + 1) * (NLR // WB)
                ]:
                    add_dep_helper(rd.ins, wi.ins, reason="W readback after write")
                o4 = None
                for ii in range(NLR):
                    i = rb * NLR + ii
                    psv = pv_psum.tile([P, H], F32, tag="pv")
                    for si in range(NS):
                        nc.tensor.matmul(
                            psv[:],
                            lhsT=wt_h[:, si, ii * P : (ii + 1) * P],
                            rhs=v_sb[:, si, :],
                            start=(si == 0),
                            stop=(si == NS - 1),
                        )
                    if ii % OB == 0:
                        o4 = o_pool.tile([P, OB, H], F32, tag="osb")
                    nc.vector.tensor_copy(o4[:, ii % OB, :], psv[:])
                    if ii % OB == OB - 1:
                        blk = i // OB
                        nc.sync.dma_start(
                            attn_out[
                                h, blk * OB * P : (blk + 1) * OB * P, :
                            ].rearrange("(a p) x -> p a x", p=P),
                            o4[:],
                        )


def build_nc(HPC, L, S, H, n_cores=N_CORES):
    nc = bacc.Bacc(
        "TRN2", target_bir_lowering=False, debug=False, num_devices=n_cores
    )
    qt = nc.declare_dram_parameter("qt", [HPC, P, L], BF16, isOutput=False)
    kt = nc.declare_dram_parameter("kt", [HPC, P, S], BF16, isOutput=False)
    v = nc.declare_dram_parameter("v", [HPC, P, (S // P) * H], BF16, isOutput=False)
    hm = nc.declare_dram_parameter("hm", [P, HPC], F32, isOutput=False)
    w_out = nc.declare_dram_parameter("w_out", [HPC, L, S], BF16, isOutput=True)
    attn_out = nc.declare_dram_parameter("attn_out", [HPC, L, H], F32, isOutput=True)
    with tile.TileContext(nc) as tc:
        emit_attention(tc, qt, kt, v, hm, w_out, attn_out, HPC, L, S, H)
    nc.compile()
    return nc


def make_in_maps(Q, K, V, mask_out, head_mask, n_cores=N_CORES):
    bf16 = ml_dtypes.bfloat16
    B, NH, L, H = Q.shape
    S = K.shape[2]
    NS = S // P
    NHtot = B * NH
    HPC = NHtot // n_cores
    sH = np.float32(np.sqrt(H))

    QT = np.zeros((B, NH, P, L), dtype=bf16)
    QT[:, :, :H, :] = Q.transpose(0, 1, 3, 2)
    QT[:, :, H, :] = sH
    KT = np.zeros((B, NH, P, S), dtype=bf16)
    KT[:, :, :H, :] = K.transpose(0, 1, 3, 2)
    maskrow = np.where(
        np.asarray(mask_out).reshape(B, 1, S), np.float32(NEG), np.float32(0.0)
    )
    KT[:, :, H, :] = maskrow  # broadcast over NH
    # V shuffled so the device tile [128, NS, H] loads contiguously:
    # vdev[h, p, si*H + x] = V[h, si*128 + p, x]
    Vb = np.asarray(V).astype(bf16).reshape(NHtot, NS, P, H)
    Vdev = np.ascontiguousarray(Vb.transpose(0, 2, 1, 3)).reshape(
        NHtot, P, NS * H
    )
    hm_full = np.ascontiguousarray(
        np.broadcast_to(
            np.asarray(head_mask, dtype=np.float32).reshape(1, NHtot), (P, NHtot)
        )
    )

    QTf = QT.reshape(NHtot, P, L)
    KTf = KT.reshape(NHtot, P, S)
    in_maps = []
    for c in range(n_cores):
        sl = slice(c * HPC, (c + 1) * HPC)
        in_maps.append(
            {
                "qt": np.ascontiguousarray(QTf[sl]),
                "kt": np.ascontiguousarray(KTf[sl]),
                "v": np.ascontiguousarray(Vdev[sl]),
                "hm": np.ascontiguousarray(hm_full[:, sl]),
            }
        )
    return in_maps, HPC


def assemble_outputs(results, B, NH, L, S, H):
    NHtot = B * NH
    n_cores = len(results)
    HPC = NHtot // n_cores
    attention_weight = np.empty((NHtot, L, S), dtype=np.float32)
    attention_out = np.empty((NHtot, L, H), dtype=np.float32)
    for c, out in enumerate(results):
        sl = slice(c * HPC, (c + 1) * HPC)
        attention_weight[sl] = np.asarray(out["w_out"]).astype(np.float32)
        attention_out[sl] = np.asarray(out["attn_out"]).astype(np.float32)
    return (
        attention_out.reshape(B, NH, L, H),
        attention_weight.reshape(B, NH, L, S),
    )


def run(Q, K, V, mask_out, head_mask, trace=False):
    Q = np.asarray(Q, dtype=np.float32)
    K = np.asarray(K, dtype=np.float32)
    V = np.asarray(V, dtype=np.float32)
    B, NH, L, H = Q.shape
    S = K.shape[2]
    in_maps, HPC = make_in_maps(Q, K, V, mask_out, head_mask)
    nc = build_nc(HPC, L, S, H)
    res = run_bass_kernel_spmd(nc, in_maps, list(range(N_CORES)), trace=trace)
    out = assemble_outputs(res.results, B, NH, L, S, H)
    return out, res


def kernel(Q, K, V, mask_out, head_mask):
    out, _ = run(Q, K, V, mask_out, head_mask)
    return out
